# revision 24
# baseline (speedup 1.0000x reference)
"""Grouped-Query Attention kernel for 8 Trainium2 NeuronCores.

Reference model: x[1,2048,2048] -> Q(32 heads x 64) / K,V(8 kv heads x 64),
per-head RMS-norm(Q,K) + RoPE, causal softmax attention, out-projection.

Sharding (tensor-parallel over heads): core c owns Q heads 4c..4c+3 and KV
head c (exactly its GQA group) and W_out rows [256c : 256c+256).  Each core
computes a full-shape partial output; the host sums the 8 partials (the
unshard step for a row-sharded W_out).

On-core strategy (v3):
  - all matmuls run in bf16 (1 PE cycle/row vs 4 for fp32); PSUM
    accumulation stays fp32, so only operand quantization (~0.4%) is lost,
    well inside the 2e-2 gate
  - the host ships x already transposed (xT[D,T] bf16): no on-chip x
    transposes at all
  - Q/K/V projections are one fused weight [D, 384] -> one PSUM accum chain
  - scores are built TRANSPOSED (S^T[j,i] = k_j . q_i) so that
      * PV needs no attention-matrix transpose:
          ctx^T[d,i] = sum_j v[j,d] * exp(S^T)[j,i]
      * the softmax denominator comes free via an extra ones-column in V
  - RMS-norm of q/k bounds |scores/8| <= 8, so exp() without max-subtraction
    is safe; masked entries are zeroed after exp via multiplicative bf16
    patterns on DVE (causal-diagonal tiles dedup to 4 unique patterns)
  - exp() runs on [128, 1024] tiles (two j-blocks share one activation
    instruction) to amortize the ACT per-instruction overhead
  - phase 2 loops i-blocks OUTER and fuses softmax-normalize + output
    projection + out-DMA per i-block, so PE/ACT/DVE/DMA overlap instead of
    running as serial phases
  - rms-norm+rope math in bf16 on DVE (2x mode), k's chain on gpsimd;
    norm scale is applied after rope (rope is linear, scale is a scalar)
"""

import numpy as np
import ml_dtypes

BF16 = ml_dtypes.bfloat16

T = 2048
D = 2048
NUM_HEADS = 32
NUM_KV = 8
HD = 64
N_CORES = 8
H_LOC = NUM_HEADS // N_CORES  # 4 q heads per core
EPS = 1e-6

TT = T // 128   # 16 t-tiles of 128 rows
CC = D // 128   # 16 contraction chunks
IBS = T // 512  # 4 i-blocks of 512 query positions
JBS = T // 128  # 16 j-blocks of 128 key positions

KEEP = "keep"
SKIP = "skip"


def _classify_mask(mask: np.ndarray):
    """Per (ib, jb) scoresT tile: how to apply the mask.

    Returns (status[IBS][JBS], patterns[n,128,512]) where patterns are
    multiplicative keep-masks in S^T (j, i) layout for partially-masked
    tiles (identical tiles dedup; the causal diagonal yields 4 uniques).
    """
    keep = ~mask
    status = [[KEEP] * JBS for _ in range(IBS)]
    pat_index: dict[bytes, int] = {}
    pats: list[np.ndarray] = []
    for ib in range(IBS):
        for jb in range(JBS):
            sub = keep[ib * 512:(ib + 1) * 512, jb * 128:(jb + 1) * 128]
            if sub.all():
                status[ib][jb] = KEEP
            elif not sub.any():
                status[ib][jb] = SKIP
            else:
                key = sub.tobytes()
                if key not in pat_index:
                    pat_index[key] = len(pats)
                    pats.append(sub.T.astype(np.float32))  # [128 j, 512 i]
                # kept-column range [i0, i1): queries with any kept key in
                # this tile; non-trivial range [z0, z1): queries whose row
                # is partially masked (needs the pattern multiply)
                kept_i = sub.any(axis=1)
                i0 = int(np.argmax(kept_i))
                i1 = 512 - int(np.argmax(kept_i[::-1]))
                part = ~sub.all(axis=1) & kept_i
                if part.any():
                    z0 = int(np.argmax(part))
                    z1 = 512 - int(np.argmax(part[::-1]))
                else:
                    z0 = z1 = i0
                status[ib][jb] = ("pat", pat_index[key], i0, i1, z0, z1)
    patterns = (
        np.stack(pats) if pats else np.zeros((1, 128, 512), dtype=np.float32)
    )
    return status, patterns


def _split_multiwaits(nc):
    """walrus in this container accepts only ONE sync-wait per instruction;
    hoist extra waits onto preceding same-engine NoOps (program order on the
    engine queue preserves the gating)."""
    import bass_rust
    from concourse import mybir

    n_fixed = 0
    for fn in nc.m.functions:
        for bb in fn.blocks:
            out = []
            for ins in bb.instructions:
                si = ins.sync_info
                if si is not None and si.on_wait and len(si.on_wait) > 1:
                    waits = list(si.on_wait)
                    ups = list(si.on_update) if si.on_update else []
                    for k, w in enumerate(waits[:-1]):
                        nop = mybir.InstNoOp(
                            name=f"{ins.name}-wnop{k}", ins=[], outs=[]
                        )
                        nop.engine = ins.engine
                        nop.sync_info = bass_rust.SyncInfo(
                            on_wait=[w], on_update=[]
                        )
                        out.append(nop)
                    ins.sync_info = bass_rust.SyncInfo(
                        on_wait=[waits[-1]], on_update=ups
                    )
                    n_fixed += 1
                out.append(ins)
            bb.instructions = out
    return n_fixed


def _build_program(status, n_pat):
    import concourse.bass as bass
    import concourse.mybir as mybir
    import concourse.tile as tile
    from concourse.masks import make_identity

    f32 = mybir.dt.float32
    bf16 = mybir.dt.bfloat16
    AX = mybir.AxisListType
    AF = mybir.ActivationFunctionType
    ALU = mybir.AluOpType

    nc = bass.Bass("TRN2", num_devices=N_CORES)
    xt_d = nc.declare_dram_parameter("xt", [D, T], bf16, isOutput=False)
    wqkv_d = nc.declare_dram_parameter(
        "wqkv", [D, H_LOC * HD + 2 * HD], bf16, isOutput=False
    )
    wo_d = nc.declare_dram_parameter("wo", [H_LOC * HD, D], bf16, isOutput=False)
    cosq_d = nc.declare_dram_parameter("cosq", [T, HD], bf16, isOutput=False)
    sinq_d = nc.declare_dram_parameter("sinq", [T, HD], bf16, isOutput=False)
    cosk_d = nc.declare_dram_parameter("cosk", [T, HD], bf16, isOutput=False)
    sink_d = nc.declare_dram_parameter("sink", [T, HD], bf16, isOutput=False)
    mpat_d = nc.declare_dram_parameter(
        "mpat", [n_pat, 128, 512], bf16, isOutput=False
    )
    out_d = nc.declare_dram_parameter("out", [T, D], bf16, isOutput=True)

    QKV = H_LOC * HD + 2 * HD  # 384: q 0:256, k 256:320, v 320:384

    def mmr(out, lhsT, rhs, **kw):
        nc.tensor.matmul(out, lhsT, rhs, **kw)

    with tile.TileContext(nc) as tc:
        with (
            tc.tile_pool(name="const", bufs=1) as const,
            tc.tile_pool(name="persist", bufs=1) as persist,
        ):
            ident32 = const.tile([128, 128], f32)
            make_identity(nc, ident32)
            ident = const.tile([128, 128], bf16)
            nc.vector.tensor_copy(ident, ident32)
            eps_t = const.tile([128, 1], f32)
            nc.vector.memset(eps_t, EPS)

            # persistent across phases
            qT = [persist.tile([64, T], bf16, name=f"qT{h}")
                  for h in range(H_LOC)]
            kT = persist.tile([64, T], bf16)
            # v with aux columns:
            #  v_aug  [128,TT,65]:  cols 0:64 = v, col 64 = 1  (even head of pair)
            #  v_aug2 [128,TT,128]: col 32 = 1, cols 64:128 = v (odd head of pair)
            v_aug = persist.tile([128, TT, 65], bf16)
            nc.vector.memset(v_aug[:, :, 64:65], 1.0)
            v_aug2 = persist.tile([128, TT, 128], bf16)
            nc.vector.memset(v_aug2[:, :, 0:64], 0.0)
            nc.vector.memset(v_aug2[:, :, 32:33], 1.0)
            ctxT = [persist.tile([128, T], f32, name=f"ctxT{p}") for p in range(2)]
            ctxB = [persist.tile([128, T], bf16, name=f"ctxB{p}") for p in range(2)]
            # denominators: row 64 <- even head of pair, row 32 <- odd head
            den = persist.tile([65, 2, T], f32)
            dbc = [persist.tile([128, T], f32, name=f"dbc{p}") for p in range(2)]
            # full xT, bf16 (chunk DMAs issued after the weight DMA below)
            xt_sb = persist.tile([128, CC, T], bf16)

            # ---------- phase 1: project qkv from xT, norm+rope ----------
            with (
                tc.tile_pool(name="p1w", bufs=1) as p1w,
                tc.tile_pool(name="p1t", bufs=2) as p1t,
                tc.tile_pool(name="ps1b", bufs=6, space="PSUM") as ps1b,
                tc.tile_pool(name="ps1c", bufs=2, space="PSUM") as ps1c,
            ):
                wqkv_sb = p1w.tile([128, CC, QKV], bf16)
                nc.sync.dma_start(
                    out=wqkv_sb, in_=wqkv_d.rearrange("(cc p) m -> p cc m", p=128)
                )
                ctabs = {}
                for nm, dd in (("cosq", cosq_d), ("sinq", sinq_d),
                               ("cosk", cosk_d), ("sink", sink_d)):
                    tab = p1w.tile([128, TT, HD], bf16, name=f"tab_{nm}")
                    nc.sync.dma_start(
                        out=tab, in_=dd.rearrange("(tt p) d -> p tt d", p=128)
                    )
                    ctabs[nm] = tab

                # xT chunk loads go after the (small) weight/table DMAs so
                # the first projection chains aren't gated on them
                for cc in range(CC):
                    nc.sync.dma_start(
                        out=xt_sb[:, cc, :],
                        in_=xt_d[cc * 128:(cc + 1) * 128, :]
                    )

                def _p1_post(tt, ps):
                    """v copies, rms-norm + rope, q/k transposes for tile tt"""
                    nc.scalar.activation(v_aug[:, tt, 0:64], ps[:, 320:384],
                                         AF.Copy)
                    nc.scalar.activation(v_aug2[:, tt, 64:128],
                                         ps[:, 320:384], AF.Copy)

                    # ---- q: 4 heads batched; norm+rope in bf16 on DVE.
                    # rope is linear, so the rms scale applies after it:
                    #   qr = (q*cos' + shuffle32(q)*sin') * rinv
                    q_sb = p1t.tile([128, H_LOC, HD], bf16, tag="q_sb")
                    nc.scalar.activation(
                        q_sb, ps[:, 0:256].rearrange("p (h d) -> p h d", h=H_LOC),
                        AF.Copy,
                    )
                    sq = p1t.tile([128, H_LOC, HD], bf16, tag="sq")
                    nc.vector.tensor_mul(sq, q_sb, q_sb)
                    ssum = p1t.tile([128, H_LOC, 1], f32, tag="ssum")
                    nc.vector.reduce_sum(ssum, sq, axis=AX.X)
                    rinv = p1t.tile([128, H_LOC, 1], f32, tag="rinv")
                    nc.scalar.activation(rinv, ssum, AF.Sqrt,
                                         bias=eps_t[:, 0:1], scale=1.0 / HD)
                    rinvb = p1t.tile([128, H_LOC, 1], bf16, tag="rinvb")
                    with nc.allow_low_precision(reason="rms scale, 2e-2 tol"):
                        nc.vector.reciprocal(rinvb, rinv)
                    cq = ctabs["cosq"][:, tt, :].rearrange("p (o d) -> p o d", o=1)
                    sqt = ctabs["sinq"][:, tt, :].rearrange("p (o d) -> p o d", o=1)
                    qr = p1t.tile([128, H_LOC, HD], bf16, tag="qr")
                    nc.vector.tensor_mul(
                        qr, q_sb, cq.to_broadcast([128, H_LOC, HD])
                    )
                    qrot = p1t.tile([128, H_LOC, HD], bf16, tag="qrot")
                    nc.vector.tensor_mul(
                        qrot[:, :, 0:32], q_sb[:, :, 32:64],
                        sqt[:, :, 0:32].to_broadcast([128, H_LOC, 32]),
                    )
                    nc.vector.tensor_mul(
                        qrot[:, :, 32:64], q_sb[:, :, 0:32],
                        sqt[:, :, 32:64].to_broadcast([128, H_LOC, 32]),
                    )
                    nc.vector.tensor_add(qr, qr, qrot)
                    qrb = p1t.tile([128, H_LOC, HD], bf16, tag="qrb")
                    nc.vector.tensor_mul(
                        qrb, qr, rinvb.to_broadcast([128, H_LOC, HD])
                    )

                    # ---- k on gpsimd (parallel with q's DVE chain)
                    kv = p1t.tile([128, HD], bf16, tag="kv")
                    nc.scalar.activation(kv, ps[:, 256:320], AF.Copy)
                    sk = p1t.tile([128, HD], bf16, tag="sk")
                    sksum = p1t.tile([128, 1], f32, tag="sksum")
                    nc.vector.tensor_mul(sk, kv, kv)
                    nc.vector.reduce_sum(sksum, sk, axis=AX.X)
                    krinv = p1t.tile([128, 1], f32, tag="krinv")
                    nc.scalar.activation(krinv, sksum, AF.Sqrt,
                                         bias=eps_t[:, 0:1], scale=1.0 / HD)
                    krinvb = p1t.tile([128, 1], bf16, tag="krinvb")
                    with nc.allow_low_precision(reason="rms scale, 2e-2 tol"):
                        nc.vector.reciprocal(krinvb, krinv)
                    kr = p1t.tile([128, HD], bf16, tag="kr")
                    nc.gpsimd.tensor_mul(kr, kv, ctabs["cosk"][:, tt, :])
                    krot = p1t.tile([128, HD], bf16, tag="krot")
                    nc.gpsimd.tensor_mul(
                        krot[:, 0:32], kv[:, 32:64],
                        ctabs["sink"][:, tt, 0:32],
                    )
                    nc.gpsimd.tensor_mul(
                        krot[:, 32:64], kv[:, 0:32],
                        ctabs["sink"][:, tt, 32:64],
                    )
                    nc.gpsimd.tensor_add(kr, kr, krot)
                    krb = p1t.tile([128, HD], bf16, tag="krb")
                    nc.gpsimd.tensor_mul(
                        krb, kr, krinvb.to_broadcast([128, HD])
                    )

                    # transpose q heads + k into qT / kT ([64, 640] = 1 bank)
                    psqt = ps1c.tile([64, 640], bf16, tag="psqt")
                    for h in range(H_LOC):
                        nc.tensor.transpose(
                            psqt[:, h * 128:(h + 1) * 128], qrb[:, h, :], ident
                        )
                    nc.tensor.transpose(psqt[:, 512:640], krb, ident)
                    for h in range(H_LOC):
                        dst = qT[h][:, tt * 128:(tt + 1) * 128]
                        src_ = psqt[:, h * 128:(h + 1) * 128]
                        if h < 2:
                            nc.scalar.activation(dst, src_, AF.Copy)
                        else:
                            nc.vector.tensor_copy(dst, src_)
                    nc.vector.tensor_copy(
                        kT[:, tt * 128:(tt + 1) * 128], psqt[:, 512:640]
                    )

                # waves of 6 t-tiles with cc-outer matmul order: 6 open PSUM
                # accumulators let the PE chase the xT chunk DMAs instead of
                # stalling on the last chunk for every tile
                WAVE = 6
                for w0 in range(0, TT, WAVE):
                    wave = list(range(w0, min(w0 + WAVE, TT)))
                    wps = {}
                    for tt in wave:
                        ps_w = ps1b.tile([128, QKV], f32, tag="ps")
                        wps[tt] = ps_w
                    for cc in range(CC):
                        for tt in wave:
                            mmr(wps[tt],
                                xt_sb[:, cc, tt * 128:(tt + 1) * 128],
                                wqkv_sb[:, cc, :],
                                start=(cc == 0), stop=(cc == CC - 1))
                    for tt in wave:
                        _p1_post(tt, wps[tt])

            # ---------- phase 2: i-block outer; scores -> exp -> PV,
            # then softmax-normalize + out-projection for that i-block ----
            with (
                tc.tile_pool(name="p2w", bufs=1) as p2w,
                tc.tile_pool(name="p2e", bufs=4) as p2e,
                tc.tile_pool(name="p2o", bufs=4) as p2o,
                tc.tile_pool(name="ps2s", bufs=2, space="PSUM") as ps2s,
                tc.tile_pool(name="ps2c", bufs=2, space="PSUM") as ps2c,
                tc.tile_pool(name="ps3o", bufs=2, space="PSUM") as ps3o,
                tc.tile_pool(name="p2d", bufs=2, space="DRAM") as p2d,
            ):
                mpat_sb = p2w.tile([128, n_pat, 512], bf16)
                nc.sync.dma_start(
                    out=mpat_sb, in_=mpat_d.rearrange("n p f -> p n f")
                )
                wo_sb = [p2w.tile([128, D], bf16, name=f"wo{p}") for p in range(2)]
                for p in range(2):
                    nc.sync.dma_start(
                        out=wo_sb[p], in_=wo_d[p * 128:(p + 1) * 128, :]
                    )

                inv_sqrt_d = float(1.0 / np.sqrt(HD))

                def bcast64(sl):
                    return bass.AP(
                        tensor=sl.tensor, offset=sl.offset,
                        ap=[[0, 64], [1, 512]],
                    )

                def _den_pipe(pair, isl):
                    """1/den for one head pair, DRAM bounce, partition
                    broadcast, and the softmax divide - issued as soon as
                    the pair's two heads are done so it overlaps the next
                    heads' attention."""
                    nc.vector.reciprocal(den[32:33, pair, isl],
                                         den[32:33, pair, isl])
                    nc.vector.reciprocal(den[64:65, pair, isl],
                                         den[64:65, pair, isl])
                    dscr = p2d.tile([2, 512], f32, tag="dscr")
                    nc.sync.dma_start(out=dscr[0:1, :],
                                      in_=den[64:65, pair, isl])
                    nc.sync.dma_start(out=dscr[1:2, :],
                                      in_=den[32:33, pair, isl])
                    nc.gpsimd.dma_start(
                        out=dbc[pair][0:64, isl],
                        in_=bcast64(dscr[0:1, :]),
                    )
                    nc.gpsimd.dma_start(
                        out=dbc[pair][64:128, isl],
                        in_=bcast64(dscr[1:2, :]),
                    )
                    nc.vector.tensor_mul(
                        ctxB[pair][:, isl], ctxT[pair][:, isl],
                        dbc[pair][:, isl],
                    )

                def _outproj_tile(tt, cb):
                    pso = ps3o.tile([128, 512], f32, tag="pso")
                    for pair in range(2):
                        mmr(pso,
                            ctxB[pair][:, tt * 128:(tt + 1) * 128],
                            wo_sb[pair][:, cb * 512:(cb + 1) * 512],
                            start=(pair == 0), stop=(pair == 1))
                    ot = p2o.tile([128, 512], bf16, tag="ot")
                    nc.vector.tensor_copy(ot, pso)
                    nc.sync.dma_start(
                        out=out_d[tt * 128:(tt + 1) * 128,
                                  cb * 512:(cb + 1) * 512],
                        in_=ot,
                    )

                # out-projection tiles of i-block N are emitted interleaved
                # into i-block N+1's attention: the PE fills exp-wait stalls
                # with ready outproj matmuls and only the last i-block's
                # projection trails the attention.
                pending = []
                for ib in range(IBS):
                    isl = slice(ib * 512, (ib + 1) * 512)
                    # per-tile kept/non-trivial column ranges (relative to
                    # the i-block).  ranges trim the QK/exp/PV compute to
                    # the unmasked queries; the first tile is forced full
                    # so every psc column gets its accumulation start.
                    entries = []
                    for jb in range(JBS):
                        st = status[ib][jb]
                        if st == SKIP:
                            continue
                        if st == KEEP:
                            entries.append((jb, 0, 512, None))
                        else:
                            _, idx, i0, i1, z0, z1 = st
                            entries.append((jb, i0, i1, (idx, z0, z1)))
                    if entries and entries[0][1:3] != (0, 512):
                        jb, i0, i1, pz = entries[0]
                        idx = pz[0] if pz else None
                        if idx is None:
                            st = status[ib][jb]
                            idx = st[1]
                        entries[0] = (jb, 0, 512, (idx, 0, 512))
                    for h in range(H_LOC):
                        pair, sub = divmod(h, 2)
                        psc = ps2c.tile([128, 512], f32, tag="psc")
                        if sub == 0:
                            ctx_out = psc[0:65, :]
                            lhs_of = lambda jb: v_aug[:, jb, :]
                            ctx_rows, den_row = (0, 64), 64
                        else:
                            ctx_out = psc
                            lhs_of = lambda jb: v_aug2[:, jb, :]
                            ctx_rows, den_row = (64, 128), 32
                        # two j-blocks per [128,1024] scores tile; tiles
                        # trimmed to their kept-column range, slot 1 always
                        # at column 512 (PSUM bank alignment)
                        for n0 in range(0, len(entries), 2):
                            epair = entries[n0:n0 + 2]
                            # slot k=0 sits at its natural columns [i0,i1);
                            # slot k=1 is based so its data starts at column
                            # 512 exactly (bank-aligned, and contiguous with
                            # slot 0 when slot 0 reaches column 512)
                            bases = [0]
                            if len(epair) == 2:
                                bases.append(512 - epair[1][1])
                            pss = ps2s.tile([128, 1024], f32, tag="pss")
                            for k, (jb, i0, i1, pz) in enumerate(epair):
                                b = bases[k]
                                mmr(pss[:, b + i0:b + i1],
                                    kT[:, jb * 128:(jb + 1) * 128],
                                    qT[h][:, ib * 512 + i0:ib * 512 + i1],
                                    start=True, stop=True)
                            et = p2e.tile([128, 1024], bf16, tag="et")
                            e0 = epair[0]
                            if len(epair) == 2 and e0[2] == 512:
                                w1 = epair[1][3 - 1] - epair[1][1]
                                nc.scalar.activation(
                                    et[:, e0[1]:512 + w1],
                                    pss[:, e0[1]:512 + w1],
                                    AF.Exp, scale=inv_sqrt_d)
                            else:
                                for k, (jb, i0, i1, pz) in enumerate(epair):
                                    b = bases[k]
                                    nc.scalar.activation(
                                        et[:, b + i0:b + i1],
                                        pss[:, b + i0:b + i1],
                                        AF.Exp, scale=inv_sqrt_d)
                            for k, (jb, i0, i1, pz) in enumerate(epair):
                                b = bases[k]
                                if pz is not None and pz[2] > pz[1]:
                                    idx, z0, z1 = pz
                                    nc.vector.tensor_mul(
                                        et[:, b + z0:b + z1],
                                        et[:, b + z0:b + z1],
                                        mpat_sb[:, idx, z0:z1],
                                    )
                                nn = n0 + k
                                mmr(ctx_out[:, i0:i1], lhs_of(jb),
                                    et[:, b + i0:b + i1],
                                    start=(nn == 0),
                                    stop=(nn == len(entries) - 1),
                                    skip_group_check=True)
                            if pending:
                                _outproj_tile(*pending.pop(0))
                        nc.vector.tensor_copy(
                            ctxT[pair][ctx_rows[0]:ctx_rows[1], isl],
                            psc[ctx_rows[0]:ctx_rows[1], :],
                        )
                        nc.vector.tensor_copy(
                            den[den_row:den_row + 1, pair, isl],
                            psc[den_row:den_row + 1, :],
                        )
                        if h == 1:
                            _den_pipe(0, isl)
                    _den_pipe(1, isl)
                    while pending:
                        _outproj_tile(*pending.pop(0))
                    pending = [(tt, cb) for tt in range(4 * ib, 4 * ib + 4)
                               for cb in range(4)]
                while pending:
                    _outproj_tile(*pending.pop(0))

    _split_multiwaits(nc)
    return nc


_CACHE = {}


def _get_program(mask_key, status, n_pat):
    if mask_key not in _CACHE:
        _CACHE[mask_key] = _build_program(status, n_pat)
    return _CACHE[mask_key]


def _prepare(x, mask, cos, sin, W_query, W_key, W_value, W_out,
             q_scale, k_scale):
    """Host-side prep: fold scales into rope tables, shard weights,
    classify the mask.  Returns (nc, in_maps)."""
    cos = np.asarray(cos, dtype=np.float32)
    sin = np.asarray(sin, dtype=np.float32)
    W_query = np.asarray(W_query, dtype=np.float32)
    W_key = np.asarray(W_key, dtype=np.float32)
    W_value = np.asarray(W_value, dtype=np.float32)
    W_out = np.asarray(W_out, dtype=np.float32)
    q_scale = np.asarray(q_scale, dtype=np.float32)
    k_scale = np.asarray(k_scale, dtype=np.float32)
    mask = np.asarray(mask)

    xt = np.ascontiguousarray(
        np.asarray(x).reshape(T, D).astype(np.float32).T
    ).astype(BF16)

    # rope = q*cos' + shuffle32(q)*sin' with the rotate-half signs and the
    # post-norm q/k scales folded into the tables:
    #   rope(s*qn) = qn*(s*cos) + shuffle32(qn)*(shuffle32(s)*sin+-)
    def tables(scale):
        perm = np.concatenate([scale[HD // 2:], scale[:HD // 2]])
        c = (cos * scale[None, :]).astype(np.float32)
        s = (sin * perm[None, :]).astype(np.float32)
        s[:, :HD // 2] *= -1.0
        return (np.ascontiguousarray(c).astype(BF16),
                np.ascontiguousarray(s).astype(BF16))

    cq, sq_t = tables(q_scale)
    ck, sk_t = tables(k_scale)

    status, patterns = _classify_mask(mask)
    patterns_bf = patterns.astype(BF16)
    nc = _get_program(mask.tobytes(), status, patterns.shape[0])

    in_maps = []
    for c in range(N_CORES):
        qcols = slice(c * H_LOC * HD, (c + 1) * H_LOC * HD)
        kvcols = slice(c * HD, (c + 1) * HD)
        wqkv = np.concatenate(
            [W_query[:, qcols], W_key[:, kvcols], W_value[:, kvcols]], axis=1
        ).astype(BF16)
        in_maps.append({
            "xt": xt,
            "wqkv": np.ascontiguousarray(wqkv),
            "wo": np.ascontiguousarray(W_out[qcols, :].astype(BF16)),
            "cosq": cq, "sinq": sq_t, "cosk": ck, "sink": sk_t,
            "mpat": patterns_bf,
        })
    return nc, in_maps


def kernel(x, mask, cos, sin, W_query, W_key, W_value, W_out,
           q_scale, k_scale):
    out_dtype = np.asarray(x).dtype
    nc, in_maps = _prepare(x, mask, cos, sin, W_query, W_key, W_value,
                           W_out, q_scale, k_scale)

    from concourse.bass_utils import run_bass_kernel_spmd

    res = run_bass_kernel_spmd(nc, in_maps, list(range(N_CORES)))
    acc = res.results[0]["out"].astype(np.float32)
    for c in range(1, N_CORES):
        acc = acc + res.results[c]["out"].astype(np.float32)
    return acc.reshape(1, T, D).astype(out_dtype)
